# revision 1
# baseline (speedup 1.0000x reference)
"""GQA multi-head attention (B=1, S=4096, E=2048, H=16, HK=4, D=128) on 8 trn2
NeuronCores.

Sharding: tensor-parallel over query heads — 2 q-heads per core, each core
also computes the kv head its q-heads attend to (each kv head is replicated
on the 2 cores that need it). Each core produces a partial output
y_c = attn_c @ Wo_c and the host sums the 8 partials during unsharding
(so the device program needs no collectives).

Device-side dataflow per core (matmul inputs fp16, accumulation fp32):
  xT [E,S] -> qT [D,h,S], kT [D,S] (transposed projections), v [S,D]
  scoresT[t,sq] = (kT chunk as lhsT).T @ qT      (t-chunk on partitions)
  pT = exp(scoresT/sqrt(D)) via ACT -> fp16
  outT[d,sq] accumulated over t-chunks: lhsT=v[t,d], rhs=pT[t,sq]
  rowsums: DVE adds over t-chunks, then ones-matmul partition-sum+broadcast
  attnT = outT * (1/rowsum); o_proj: y[s,e] = (attnT as lhsT).T @ WoT
"""
import math
import numpy as np
from contextlib import ExitStack

import concourse.bass as bass
import concourse.mybir as mybir
from concourse import tile
from concourse import bass_utils
from concourse.masks import make_identity

B, S, E = 1, 4096, 2048
H, HK, D = 16, 4, 128
N_CORES = 8
HPC = H // N_CORES          # q heads per core
QDIM = HPC * D              # 256
EC = E // 128               # e-chunks
SB = 512                    # s/sq block
NSB = S // SB
TC = S // 128               # t-chunks
SCALE = 1.0 / math.sqrt(D)
FP16 = mybir.dt.float16
FP32 = mybir.dt.float32


def _split_sync_waits(nc, cap=1):
    """This container's walrus build rejects instructions carrying more than
    ~1 sync-wait (codegen 'Too many sync wait commands'). Post-pass over the
    scheduled BIR: for any instruction with >cap waits, hoist the excess onto
    same-engine NOPs inserted immediately before it (same block, so per-engine
    program order — and therefore semantics — is preserved)."""
    n = 0
    for fn in nc.m.functions:
        for blk in fn.blocks:
            il = blk.instructions
            i = 0
            while i < len(il):
                inst = il[i]
                si = getattr(inst, "sync_info", None)
                if si is not None and len(si.on_wait) > cap:
                    waits = list(si.on_wait)
                    si.on_wait = waits[-cap:]
                    extras = []
                    for w in waits[:-cap]:
                        nop = mybir.InstNoOp(name=f"I-waitfix-{n}", ins=[], outs=[])
                        n += 1
                        nop.engine = inst.engine
                        nop.sync_info = mybir.SyncInfo(on_wait=[w], on_update=[])
                        extras.append(nop)
                    il[i:i] = extras
                    i += len(extras)
                i += 1
    return n


XTW = 2048                  # xt tile width (half of S per tile)
NHALF = S // XTW            # 2 halves


def _emit_program(nc, tc, aps, weights, r):
    """Emit one full forward pass. `r` suffixes pool/tile names so the
    program can be repeated for timing calibration.

    The PE executes its instruction stream in order, so emission order is
    the PE schedule. Phase A streams x from HBM exactly once, computing the
    k/q/v projections interleaved from the same resident x tiles; v is
    produced transposed (wide 512-col matmuls) then flipped into [t, d]
    layout with PE transposes against an identity (one shared PSUM tile,
    single evict) instead of 128-col ldweights-paced matmuls. The attention
    inner loop is software-pipelined (QK of chunk tp+1 is emitted before PV
    of chunk tp, so the PE never waits on the exp of the chunk it just
    produced), and independent matmuls (output projection of the previous
    query block, rowsum matmuls) are injected as fillers into the slack the
    ACT-paced exp stream leaves on the PE.

    PSUM budget (8 banks): kv/y shared tag 1, q 1, sps 2x2, osum 2x1."""
    xT, y = aps
    wq_sb, wk_sb, wv_sb, wo_sb, ones_sb, ident_sb = weights

    big = tc.alloc_tile_pool(name=f"big{r}", bufs=1)
    qT_sb = big.tile([128, HPC, S], FP16, name=f"qT{r}")   # [d, h, s]
    kT_sb = big.tile([128, S], FP16, name=f"kT{r}")        # [d, t]
    v_sb = big.tile([128, S], FP16, name=f"v{r}")          # [t%128, tc*128+d]
    aT_sb = big.tile([128, HPC, S], FP16, name=f"aT{r}")   # [d, h, s]

    with ExitStack() as ctx:
        xpool = ctx.enter_context(tc.tile_pool(name=f"xpool{r}", bufs=24))
        ps = ctx.enter_context(tc.tile_pool(name=f"ps{r}", bufs=1, space="PSUM"))
        ptp = ctx.enter_context(tc.tile_pool(name=f"ptp{r}", bufs=6))
        accp = ctx.enter_context(tc.tile_pool(name=f"accp{r}", bufs=3))
        y_sbp = ctx.enter_context(tc.tile_pool(name=f"y_sbp{r}", bufs=2))
        vtp = ctx.enter_context(tc.tile_pool(name=f"vtp{r}", bufs=2))

        xt_tiles = {}

        def load_half(half):
            for ec in range(EC):
                t = xpool.tile([128, XTW], FP16,
                               name=f"xt{r}_{half}_{ec}", tag="xt")
                nc.sync.dma_start(
                    t[:], xT[ec * 128:(ec + 1) * 128,
                             half * XTW:(half + 1) * XTW])
                xt_tiles[(half, ec)] = t

        def xt_slice(sb, ec, width=SB, sub=0):
            half, off = divmod(sb * SB + sub, XTW)
            return xt_tiles[(half, ec)][:, off:off + width]

        def q_group(qb, ic):
            """Emit one q-projection accumulation group (16 matmuls+evict)."""
            q_ps = ps.tile([128, SB], FP32, name=f"qps{r}_{qb}_{ic}", tag="q")
            for ec in range(EC):
                nc.tensor.matmul(
                    q_ps[:],
                    wq_sb[:, ec, ic * 128:(ic + 1) * 128],
                    xt_slice(qb, ec),
                    start=(ec == 0), stop=(ec == EC - 1))
            nc.vector.tensor_copy(
                qT_sb[:, ic, qb * SB:(qb + 1) * SB], q_ps[:])

        # ---- Phase A: k/v/q projections (x streamed from HBM exactly once).
        # First-use-ordered DMAs: interleave wk/wv with the first xt tiles so
        # the first k matmul starts within a few us, then stream the rest.
        if r == 0:
            wdmas = _WEIGHT_DMAS.pop(0)
            kv_dmas = wdmas["kv"]       # list of (dst_ap, src_ap)
            q_dmas = wdmas["q"]
            late_dmas = wdmas["o"]
        else:
            kv_dmas, q_dmas, late_dmas = [], [], []
        for ec in range(EC):
            picks = ([kv_dmas[0]] if ec == 0 and kv_dmas else
                     [kv_dmas[1]] if ec == 2 and kv_dmas else [])
            for dst, src in picks:
                nc.sync.dma_start(dst, src)
            t = xpool.tile([128, XTW], FP16, name=f"xt{r}_0_{ec}", tag="xt")
            nc.sync.dma_start(t[:], xT[ec * 128:(ec + 1) * 128, 0:XTW])
            xt_tiles[(0, ec)] = t
        for dst, src in q_dmas:
            nc.sync.dma_start(dst, src)
        for sbp in range(NSB // 2):
            if sbp == 1:
                load_half(1)
            if sbp == 3:
                for dst, src in late_dmas:
                    nc.sync.dma_start(dst, src)
            sb0, sb1 = 2 * sbp, 2 * sbp + 1
            k_ps = ps.tile([128, 2 * SB], FP32, name=f"kps{r}_{sbp}",
                           tag="sps", bufs=2)
            for ec in range(EC):
                nc.tensor.matmul(k_ps[:, 0:SB], wk_sb[:, ec, :],
                                 xt_slice(sb0, ec),
                                 start=(ec == 0), stop=(ec == EC - 1))
                nc.tensor.matmul(k_ps[:, SB:2 * SB], wk_sb[:, ec, :],
                                 xt_slice(sb1, ec),
                                 start=(ec == 0), stop=(ec == EC - 1))
            nc.vector.tensor_copy(
                kT_sb[:, sb0 * SB:(sb0 + 2) * SB], k_ps[:])
            if sbp < 2:
                q_group(sb0, 0)
            vT_ps = ps.tile([128, 2 * SB], FP32, name=f"vtps{r}_{sbp}",
                            tag="sps", bufs=2)
            for ec in range(EC):
                nc.tensor.matmul(vT_ps[:, 0:SB], wv_sb[:, ec, :],
                                 xt_slice(sb0, ec),
                                 start=(ec == 0), stop=(ec == EC - 1))
                nc.tensor.matmul(vT_ps[:, SB:2 * SB], wv_sb[:, ec, :],
                                 xt_slice(sb1, ec),
                                 start=(ec == 0), stop=(ec == EC - 1))
            vT_st = vtp.tile([128, 2 * SB], FP16, name=f"vst{r}_{sbp}",
                             tag="vst")
            nc.scalar.copy(vT_st[:], vT_ps[:])
            if sbp < 2:
                q_group(sb0, 1)
            vt_ps = ps.tile([128, 2 * SB], FP16, name=f"vtt{r}_{sbp}",
                            tag="sps", bufs=2)
            for j in range(2 * SB // 128):
                nc.tensor.transpose(vt_ps[:, j * 128:(j + 1) * 128],
                                    vT_st[:, j * 128:(j + 1) * 128],
                                    ident_sb[:])
            nc.scalar.copy(v_sb[:, sb0 * SB:(sb0 + 2) * SB], vt_ps[:])
            if sbp < 2:
                q_group(sb1, 0)
                q_group(sb1, 1)

        n_y = [0]

        def o_proj_closures(qb, tag="kv"):
            """One closure per (sc, eb): 2 matmuls + evict; store per sc."""
            cls = []
            for sc in range(qb * (SB // 128), (qb + 1) * (SB // 128)):
                y_t = y_sbp.tile([128, E], FP16, name=f"ysb{r}_{sc}",
                                 tag="ysb")
                for eb in range(E // SB):
                    def mk(sc, eb, y_t):
                        def emit():
                            shape = [128, 2 * SB] if tag == "sps" else [128, SB]
                            y_ps = ps.tile(shape, FP32,
                                           name=f"yps{r}_{sc}_{eb}", tag=tag,
                                           bufs=2 if tag == "sps" else None)
                            for h in range(HPC):
                                nc.tensor.matmul(
                                    y_ps[:, 0:SB],
                                    aT_sb[:, h, sc * 128:(sc + 1) * 128],
                                    wo_sb[:, h, eb * SB:(eb + 1) * SB],
                                    start=(h == 0), stop=(h == HPC - 1))
                            nc.vector.tensor_copy(
                                y_t[:, eb * SB:(eb + 1) * SB], y_ps[:, 0:SB])
                            n_y[0] += 1
                            if eb == E // SB - 1:
                                nc.sync.dma_start(
                                    y[sc * 128:(sc + 1) * 128, :], y_t[:])
                        return emit
                    cls.append(mk(sc, eb, y_t))
            return cls

        def finish_head_closure(h, qb, o_ps, sums):
            def emit():
                sums_ps = ps.tile([128, SB], FP32,
                                  name=f"sums_ps{r}_{h}_{qb}", tag="q")
                nc.tensor.matmul(sums_ps[:], ones_sb[:], sums[:],
                                 start=True, stop=True)
                recip = accp.tile([128, SB], FP32, name=f"recip{r}_{h}_{qb}",
                                  tag="recip")
                nc.vector.reciprocal(recip[:], sums_ps[:])
                nc.vector.tensor_mul(
                    aT_sb[:, h, qb * SB:(qb + 1) * SB], o_ps[:], recip[:])
            return emit

        def q_proj_closures(qb):
            """One closure per matmul of q(qb); evict attached to the last."""
            cls = []
            for ic in range(HPC):
                q_ps = ps.tile([128, SB], FP32, name=f"qps{r}_{qb}_{ic}",
                               tag="q")

                def mk(ic, ec, q_ps):
                    def emit():
                        nc.tensor.matmul(
                            q_ps[:],
                            wq_sb[:, ec, ic * 128:(ic + 1) * 128],
                            xt_slice(qb, ec),
                            start=(ec == 0), stop=(ec == EC - 1))
                        if ec == EC - 1:
                            nc.vector.tensor_copy(
                                qT_sb[:, ic, qb * SB:(qb + 1) * SB], q_ps[:])
                    return emit
                for ec in range(EC):
                    cls.append(mk(ic, ec, q_ps))
            return cls

        # ---- Phase B: attention, with q(4..7)/o-projection as PE fillers ----
        from collections import deque
        fillers = deque()
        for qb_late in range(NSB // 2, NSB):
            fillers.extend(q_proj_closures(qb_late))
        pending_oproj = []       # o_proj of qb becomes available after qb

        for qb in range(NSB):
            fillers.extend(pending_oproj)
            pending_oproj = []
            for h in range(HPC):
                o_ps = ps.tile([128, SB], FP32, name=f"ops{r}_{h}_{qb}",
                               tag="osum", bufs=2)
                acc2 = accp.tile([128, 2 * SB], FP16, name=f"acc{r}_{h}_{qb}",
                                 tag="acc")

                def qk(tp):
                    s_ps = ps.tile([128, 2 * SB], FP32,
                                   name=f"sps{r}_{h}_{qb}_{tp}", tag="sps",
                                   bufs=2)
                    for hf in range(2):
                        t = tp * 2 + hf
                        nc.tensor.matmul(
                            s_ps[:, hf * SB:(hf + 1) * SB],
                            kT_sb[:, t * 128:(t + 1) * 128],
                            qT_sb[:, h, qb * SB:(qb + 1) * SB],
                            start=True, stop=True)
                    return s_ps

                s_prev = qk(0)
                pt_first = None
                for tp in range(TC // 2):
                    pt = ptp.tile([128, 2 * SB], FP16,
                                  name=f"pt{r}_{h}_{qb}_{tp}", tag="pt")
                    nc.scalar.activation(
                        pt[:], s_prev[:],
                        mybir.ActivationFunctionType.Exp, scale=SCALE)
                    if tp + 1 < TC // 2:
                        s_prev = qk(tp + 1)
                    for hf in range(2):
                        t = tp * 2 + hf
                        nc.tensor.matmul(
                            o_ps[:],
                            v_sb[:, t * 128:(t + 1) * 128],
                            pt[:, hf * SB:(hf + 1) * SB],
                            start=(t == 0), stop=(t == TC - 1))
                    if tp == 0:
                        pt_first = pt
                    elif tp == 1:
                        nc.vector.tensor_add(acc2[:], pt_first[:], pt[:])
                    else:
                        nc.vector.tensor_add(acc2[:], acc2[:], pt[:])
                    slots_left = (HPC - h) * (TC // 2) - tp
                    n_pop = min(len(fillers),
                                max(1, -(-len(fillers) // max(slots_left, 1))),
                                3)
                    for _ in range(n_pop):
                        if fillers:
                            fillers.popleft()()
                sums = accp.tile([128, SB], FP16, name=f"sums{r}_{h}_{qb}",
                                 tag="sums")
                nc.vector.tensor_add(sums[:], acc2[:, 0:SB], acc2[:, SB:2 * SB])
                fillers.append(finish_head_closure(h, qb, o_ps, sums))
            pending_oproj = o_proj_closures(
                qb, tag="sps" if qb == NSB - 1 else "kv")

        while fillers:
            fillers.popleft()()
        for c in pending_oproj:
            c()

    big.release()


_WEIGHT_DMAS = []


def build_bass(reps=1):
    nc = bass.Bass("TRN2", target_bir_lowering=False, debug=False,
                   num_devices=N_CORES)
    xT = nc.dram_tensor("xT", [E, S], FP16, kind="ExternalInput").ap()
    wq = nc.dram_tensor("wq", [E, QDIM], FP16, kind="ExternalInput").ap()
    wk = nc.dram_tensor("wk", [E, D], FP16, kind="ExternalInput").ap()
    wv = nc.dram_tensor("wv", [E, D], FP16, kind="ExternalInput").ap()
    wo = nc.dram_tensor("wo", [QDIM, E], FP16, kind="ExternalInput").ap()
    y = nc.dram_tensor("y", [S, E], FP16, kind="ExternalOutput").ap()

    with tile.TileContext(nc) as tc, ExitStack() as ctx:
        wpool = ctx.enter_context(tc.tile_pool(name="wpool", bufs=1))
        wq_sb = wpool.tile([128, EC, QDIM], FP16)
        wk_sb = wpool.tile([128, EC, D], FP16)
        wv_sb = wpool.tile([128, EC, D], FP16)
        wo_sb = wpool.tile([128, HPC, E], FP16)
        ones_sb = wpool.tile([128, 128], FP16)
        nc.vector.memset(ones_sb[:], 1.0)
        ident_sb = wpool.tile([128, 128], FP16)
        make_identity(nc, ident_sb)
        kv_dmas = [
            (wk_sb[:], wk.rearrange("(ec p) d -> p ec d", p=128)),
            (wv_sb[:], wv.rearrange("(ec p) d -> p ec d", p=128)),
        ]
        q_dmas = [
            (wq_sb[:], wq.rearrange("(ec p) d -> p ec d", p=128)),
        ]
        o_dmas = [
            (wo_sb[:], wo.rearrange("(h p) e -> p h e", p=128)),
        ]
        _WEIGHT_DMAS.clear()
        _WEIGHT_DMAS.append({"kv": kv_dmas, "q": q_dmas, "o": o_dmas})

        for r in range(reps):
            _emit_program(nc, tc, (xT, y), (wq_sb, wk_sb, wv_sb, wo_sb, ones_sb, ident_sb), r)

    _split_sync_waits(nc)
    return nc


def make_in_maps(x, Wq, Wk, Wv, Wo):
    """Host-side sharding: transpose/cast to fp16, slice weights per core."""
    x = np.asarray(x, dtype=np.float32).reshape(S, E)
    xT = np.ascontiguousarray(x.T).astype(np.float16)
    WqT = np.ascontiguousarray(np.asarray(Wq, dtype=np.float32).T).astype(np.float16)
    WkT = np.ascontiguousarray(np.asarray(Wk, dtype=np.float32).T).astype(np.float16)
    WvT = np.ascontiguousarray(np.asarray(Wv, dtype=np.float32).T).astype(np.float16)
    WoT = np.ascontiguousarray(np.asarray(Wo, dtype=np.float32).T).astype(np.float16)
    in_maps = []
    for c in range(N_CORES):
        g = (c * HPC) // (H // HK)      # kv head for this core's q heads
        in_maps.append({
            "xT": xT,
            "wq": np.ascontiguousarray(WqT[:, c * QDIM:(c + 1) * QDIM]),
            "wk": np.ascontiguousarray(WkT[:, g * D:(g + 1) * D]),
            "wv": np.ascontiguousarray(WvT[:, g * D:(g + 1) * D]),
            "wo": np.ascontiguousarray(WoT[c * QDIM:(c + 1) * QDIM, :]),
        })
    return in_maps


_NC_CACHE = None


def get_nc():
    global _NC_CACHE
    if _NC_CACHE is None:
        _NC_CACHE = build_bass()
    return _NC_CACHE


def kernel(x, Wq, Wk, Wv, Wo):
    nc = get_nc()
    in_maps = make_in_maps(x, Wq, Wk, Wv, Wo)
    res = bass_utils.run_bass_kernel_spmd(
        nc, in_maps, core_ids=list(range(N_CORES)))
    out = np.zeros((S, E), dtype=np.float32)
    for r in res.results:
        out += r["y"].astype(np.float32)
    return out.reshape(B, S, E)

